# revision 3
# baseline (speedup 1.0000x reference)
"""Bass/Trainium2 kernel for nn_Channel_attention (bottom-16 channel gather).

reference semantics (per sample b):
    weight = mean(x[b], axis=(H, W))           # [C]
    idx    = argsort(weight)[:16]              # ascending pooled value
    out[b] = x[b, idx]                         # [16, H, W]

Strategy: pure data parallel, B=16 sharded 2 samples per core over 8 cores.
Per core (x shard viewed as [512, 16384] = [(sample, channel), H*W]):

  Sample 0 streams through a small rotating pool ([128, 4096] tiles, 2 MiB
  per DMA); DVE reduce per chunk -> per-channel sums -> negate -> PE
  transpose -> two max8/max_index rounds give the bottom-16 channel
  indices in ascending order.  The 16 channels are then re-fetched from
  HBM with one [128, 2048] SWDGE indirect gather (16 ch x 8 sub-rows) and
  stored with one direct DMA; both overlap sample 1's streaming.

  Sample 1 streams into two RESIDENT [128, 16384] SBUF tiles (one per
  128-channel half, 16 MiB total), so its loads have no buffer-recycle
  backpressure and the data is still on-chip when the selection finishes.
  The output is then written with two SWDGE indirect SCATTERS (SBUF ->
  DRAM, out_offset = output rank for selected channels, OOB-skipped
  otherwise), which avoids the 1 MiB gather re-read and one DMA hop on
  the end-of-kernel critical path.  The last half's final chunks shrink
  (2048/1024-wide) so the last reduce exits quickly after the last load.

  Sample 1's load issues are emitted before sample 0's gather-dependent
  store in the sync/scalar queue order, so the (waiting) store never
  stalls load descriptor generation.
"""

import sys

if "/opt/trn_rl_repo" not in sys.path:
    sys.path.insert(0, "/opt/trn_rl_repo")

import numpy as np

from concourse import bacc, mybir, tile
from concourse.bass import IndirectOffsetOnAxis
from concourse.bass_utils import run_bass_kernel_spmd
from concourse.masks import make_identity

N_CORES = 8
B, C, H, W = 16, 256, 128, 128
K = 16
BPC = B // N_CORES          # samples per core = 2
E = H * W                   # 16384 elems per channel
GR = 8                      # gather sub-rows per channel (8 x 8KiB)
ROWS = BPC * C              # 512 channel rows per core
OOB = 1024                  # scatter offset for unselected channels

f32 = mybir.dt.float32
i32 = mybir.dt.int32
u32 = mybir.dt.uint32
X = mybir.AxisListType.X
Alu = mybir.AluOpType

CHUNKS = [4096] * 4
CHUNKS_LAST = [4096, 4096, 4096, 2048, 1024, 1024]

_cache = {}


def _build():
    nc = bacc.Bacc("TRN2", target_bir_lowering=False, debug=False,
                   num_devices=N_CORES)
    x_d = nc.dram_tensor("x", [ROWS, E], f32, kind="ExternalInput")
    y_d = nc.dram_tensor("y", [BPC * K, E], f32, kind="ExternalOutput")

    with tile.TileContext(nc) as tc:
        with (
            tc.tile_pool(name="load", bufs=3) as load_pool,
            tc.tile_pool(name="small", bufs=1) as small,
            tc.tile_pool(name="psum", bufs=1, space="PSUM") as psum,
        ):
            # ---- constants (no deps; scheduler fills gaps with these) ----
            ident = small.tile([128, 128], f32)
            make_identity(nc, ident[:])

            # [16, 128] row iota 0..127 (f32) and its >>3 variant
            row_i = small.tile([K, 128], i32)
            nc.gpsimd.iota(out=row_i[:], pattern=[[1, 128]], base=0,
                           channel_multiplier=0)
            row_f = small.tile([K, 128], f32)
            nc.vector.tensor_copy(row_f[:], row_i[:])
            rowd8_i = small.tile([K, 128], i32)
            nc.vector.tensor_scalar(out=rowd8_i[:], in0=row_i[:], scalar1=3,
                                    scalar2=None, op0=Alu.arith_shift_right)
            rowd8_f = small.tile([K, 128], f32)
            nc.vector.tensor_copy(rowd8_f[:], rowd8_i[:])

            # [16, 1] partition iota (f32)
            col16_i = small.tile([K, 1], i32)
            nc.gpsimd.iota(out=col16_i[:], pattern=[[1, 1]], base=0,
                           channel_multiplier=1)
            col16_f = small.tile([K, 1], f32)
            nc.vector.tensor_copy(col16_f[:], col16_i[:])

            # onehot16[k, p] = (p>>3 == k), used to expand 16 gather rows
            onehot16 = small.tile([K, 128], f32)
            nc.vector.tensor_scalar(out=onehot16[:], in0=rowd8_f[:],
                                    scalar1=col16_f[:], scalar2=None,
                                    op0=Alu.is_equal)

            # [128, 1] (p & 7) as f32, for gather sub-row offsets
            pp = small.tile([128, 1], i32)
            nc.gpsimd.iota(out=pp[:], pattern=[[1, 1]], base=0,
                           channel_multiplier=1)
            nc.vector.tensor_scalar(out=pp[:], in0=pp[:], scalar1=GR - 1,
                                    scalar2=None, op0=Alu.bitwise_and)
            a7f = small.tile([128, 1], f32)
            nc.vector.tensor_copy(a7f[:], pp[:])

            # rank column for scatter offsets: k + s*K - OOB (s=1)
            rk_f = small.tile([K, 1], f32)
            nc.vector.tensor_scalar(out=rk_f[:], in0=col16_f[:],
                                    scalar1=float(K - OOB), scalar2=None,
                                    op0=Alu.add)

            xg = x_d[:].rearrange("r (u e) -> (r u) e", u=GR)
            dma_engines = [nc.sync, nc.scalar]
            state = {"n_dma": 0}

            # resident tiles for sample 1 (one per 128-channel half)
            big0 = small.tile([128, E], f32, tag="big0")
            big1 = small.tile([128, E], f32, tag="big1")
            big = [big0, big1]

            def stream_sample(s, into_resident):
                """Emit loads + per-chunk reduces for sample s. Returns the
                negated per-channel sums tile [1, 256] in SBUF."""
                chunk_lists = []
                for h in range(2):
                    cl = (CHUNKS_LAST if (s == BPC - 1 and h == 1)
                          else CHUNKS)
                    chunk_lists.append(cl)
                ncols = max(len(cl) for cl in chunk_lists)
                partials = small.tile([128, 2 * ncols], f32,
                                      tag=f"partials{s}")
                sums = small.tile([128, 2], f32, tag=f"sums{s}")
                psum_w = psum.tile([1, C], f32, tag=f"psw{s}")
                w_neg = small.tile([1, C], f32, tag=f"wneg{s}")
                for h in range(2):
                    base = s * C + h * 128
                    off = 0
                    cl = chunk_lists[h]
                    for j, cw in enumerate(cl):
                        if into_resident:
                            dst = big[h][:, off:off + cw]
                        else:
                            t = load_pool.tile([128, 4096], f32)
                            dst = t[:, 0:cw]
                        eng = dma_engines[state["n_dma"] % 2]
                        state["n_dma"] += 1
                        eng.dma_start(out=dst,
                                      in_=x_d[base:base + 128, off:off + cw])
                        nc.vector.reduce_sum(
                            out=partials[:, h * ncols + j:h * ncols + j + 1],
                            in_=dst, axis=X)
                        off += cw
                    # per-half: negated sums + PE transpose into [1, 256]
                    nc.vector.reduce_sum(
                        out=sums[:, h:h + 1],
                        in_=partials[:, h * ncols:h * ncols + len(cl)],
                        axis=X, negate=True)
                    nc.tensor.matmul(out=psum_w[:, h * 128:(h + 1) * 128],
                                     lhsT=sums[:, h:h + 1], rhs=ident[:],
                                     start=True, stop=True)
                    nc.vector.tensor_copy(w_neg[:, h * 128:(h + 1) * 128],
                                          psum_w[:, h * 128:(h + 1) * 128])
                return w_neg

            def select16(s, w_neg):
                """Two max8 rounds on -sums -> idx_t [16, 1] f32 in SBUF
                (bottom-16 channel indices, ascending pooled sum)."""
                m1 = small.tile([1, 8], f32, tag=f"m1_{s}")
                m2 = small.tile([1, 8], f32, tag=f"m2_{s}")
                w_rep = small.tile([1, C], f32, tag=f"wrep{s}")
                idx_u = small.tile([1, K], u32, tag=f"idxu{s}")
                nc.vector.max(out=m1[:], in_=w_neg[:])
                nc.vector.max_index(out=idx_u[:, 0:8], in_max=m1[:],
                                    in_values=w_neg[:])
                nc.vector.match_replace(out=w_rep[:], in_to_replace=m1[:],
                                        in_values=w_neg[:], imm_value=-1e38)
                nc.vector.max(out=m2[:], in_=w_rep[:])
                nc.vector.max_index(out=idx_u[:, 8:16], in_max=m2[:],
                                    in_values=w_rep[:])
                idx_f = small.tile([1, K], f32, tag=f"idxf{s}")
                nc.vector.tensor_copy(idx_f[:], idx_u[:])
                psum_t = psum.tile([K, 1], f32, tag=f"pst{s}")
                nc.tensor.matmul(out=psum_t[:], lhsT=idx_f[:],
                                 rhs=ident[0:1, 0:1], start=True, stop=True)
                idx_t = small.tile([K, 1], f32, tag=f"idxt{s}")
                nc.vector.tensor_copy(idx_t[:], psum_t[:])
                return idx_t

            # ---------------- sample 0: stream through pool ----------------
            w0 = stream_sample(0, into_resident=False)

            # ---------------- sample 1: loads into resident tiles ----------
            # (emitted now so the sync/scalar queues issue these before the
            # gather-dependent sample-0 store)
            w1 = stream_sample(1, into_resident=True)

            # ---------------- sample 0: select + gather + store ------------
            idx_t0 = select16(0, w0)
            # gather-row index per partition p: idx[p>>3]*8 + (p&7)
            psum_g = psum.tile([128, 1], f32, tag="psg")
            nc.tensor.matmul(out=psum_g[:], lhsT=onehot16[:], rhs=idx_t0[:],
                             start=True, stop=True)
            grow_f = small.tile([128, 1], f32, tag="growf")
            nc.vector.tensor_scalar(out=grow_f[:], in0=psum_g[:],
                                    scalar1=float(GR), scalar2=None,
                                    op0=Alu.mult)
            grow_i = small.tile([128, 1], i32, tag="growi")
            nc.vector.tensor_tensor(out=grow_i[:], in0=grow_f[:],
                                    in1=a7f[:], op=Alu.add)
            g = small.tile([128, E // GR], f32, tag="g0")
            nc.gpsimd.indirect_dma_start(
                out=g[:], out_offset=None, in_=xg,
                in_offset=IndirectOffsetOnAxis(ap=grow_i[:], axis=0))
            yv0 = y_d[0:K].rearrange("r (u e) -> (r u) e", u=GR)
            nc.sync.dma_start(out=yv0, in_=g[:])

            # ---------------- sample 1: select + scatter --------------------
            idx_t1 = select16(1, w1)
            for h in range(2):
                # onehotT[k, p] = (idx[k] - 128h == p)
                oh = small.tile([K, 128], f32, tag=f"oh{h}")
                if h == 0:
                    nc.vector.tensor_scalar(out=oh[:], in0=row_f[:],
                                            scalar1=idx_t1[:], scalar2=None,
                                            op0=Alu.is_equal)
                else:
                    idx_sh = small.tile([K, 1], f32, tag="idxsh")
                    nc.vector.tensor_scalar(out=idx_sh[:], in0=idx_t1[:],
                                            scalar1=-128.0, scalar2=None,
                                            op0=Alu.add)
                    nc.vector.tensor_scalar(out=oh[:], in0=row_f[:],
                                            scalar1=idx_sh[:], scalar2=None,
                                            op0=Alu.is_equal)
                # offs[p] = K + rank(p) if selected else OOB  (s=1 rows)
                psum_o = psum.tile([128, 1], f32, tag=f"pso{h}")
                nc.tensor.matmul(out=psum_o[:], lhsT=oh[:], rhs=rk_f[:],
                                 start=True, stop=True)
                offs = small.tile([128, 1], i32, tag=f"offs{h}")
                nc.vector.tensor_scalar(out=offs[:], in0=psum_o[:],
                                        scalar1=float(OOB), scalar2=None,
                                        op0=Alu.add)
                nc.gpsimd.indirect_dma_start(
                    out=y_d[:],
                    out_offset=IndirectOffsetOnAxis(ap=offs[:], axis=0),
                    in_=big[h][:], in_offset=None,
                    bounds_check=BPC * K - 1, oob_is_err=False)

    nc.compile()
    return nc


def get_nc():
    if "nc" not in _cache:
        _cache["nc"] = _build()
    return _cache["nc"]


def make_in_maps(x: np.ndarray) -> list[dict[str, np.ndarray]]:
    x = np.ascontiguousarray(np.asarray(x, dtype=np.float32))
    assert x.shape == (B, C, H, W)
    return [{"x": x[c * BPC:(c + 1) * BPC].reshape(ROWS, E)}
            for c in range(N_CORES)]


def assemble(results: list[dict[str, np.ndarray]]) -> np.ndarray:
    out = np.empty((B, K, H, W), dtype=np.float32)
    for c in range(N_CORES):
        out[c * BPC:(c + 1) * BPC] = results[c]["y"].reshape(BPC, K, H, W)
    return out


def kernel(x: np.ndarray) -> np.ndarray:
    nc = get_nc()
    res = run_bass_kernel_spmd(nc, make_in_maps(x), list(range(N_CORES)))
    return assemble(res.results)


# revision 9
# speedup vs baseline: 1.0542x; 1.0542x over previous
"""Bass/Trainium2 kernel for nn_Channel_attention (bottom-16 channel gather).

reference semantics (per sample b):
    weight = mean(x[b], axis=(H, W))           # [C]
    idx    = argsort(weight)[:16]              # ascending pooled value
    out[b] = x[b, idx]                         # [16, H, W]

Strategy: pure data parallel, B=16 sharded 2 samples per core over 8 cores.
Per core (x shard viewed as [512, 16384] = [(sample, channel), H*W]):

  Sample 0 streams through a rotating pool ([128, 4096] tiles, 2 MiB per
  DMA); DVE reduce per chunk -> per-channel sums -> negate -> PE transpose
  -> two max8/max_index rounds give the bottom-16 channel indices in
  ascending pooled order.  The 16 channels are then re-fetched from HBM
  with one [128, 2048] SWDGE indirect gather (16 ch x 8 sub-rows) and
  stored with one direct DMA; both overlap sample 1's streaming.

  Sample 1 streams into two RESIDENT [128, 16384] SBUF tiles (16 MiB), so
  its data is still on-chip when its selection finishes.  The output is
  then written with two SWDGE indirect SCATTERS (SBUF -> DRAM,
  out_offset = output rank for selected channels, OOB-skipped otherwise),
  avoiding the gather re-read and one DMA hop on the end-of-kernel
  critical path.  The two scatters target two different DRAM tensors
  ("y" and "y2") so no write-write dependency serializes them; the host
  merges rows using the stored selection indices ("sel").  Scatter
  offsets are built per max8 round with an accumulating PE matmul so
  round 1's share of the work hides under round 2's DVE ops.

  Emission order per engine queue is load-bearing: sample 1's load
  issues go to the sync/scalar queues before sample 0's gather-dependent
  store (so the waiting store can't stall load descriptor generation),
  and sample 0's select precedes sample 1's reduces on the Vector queue
  (so sample 0's gather+store run mid-stream, not in the tail).
"""

import sys

if "/opt/trn_rl_repo" not in sys.path:
    sys.path.insert(0, "/opt/trn_rl_repo")

import numpy as np

from concourse import bacc, mybir, tile
from concourse.bass import IndirectOffsetOnAxis
from concourse.bass_utils import run_bass_kernel_spmd
from concourse.masks import make_identity

N_CORES = 8
B, C, H, W = 16, 256, 128, 128
K = 16
BPC = B // N_CORES          # samples per core = 2
E = H * W                   # 16384 elems per channel
GR = 8                      # gather sub-rows per channel (8 x 8KiB)
ROWS = BPC * C              # 512 channel rows per core
OOB = 1024                  # scatter offset for unselected channels

f32 = mybir.dt.float32
i32 = mybir.dt.int32
u32 = mybir.dt.uint32
X = mybir.AxisListType.X
Alu = mybir.AluOpType

CHUNKS = [4096] * 4
CHUNKS_LAST = [4096, 4096, 4096, 2048, 1024, 512, 512]

_cache = {}


def _build():
    nc = bacc.Bacc("TRN2", target_bir_lowering=False, debug=False,
                   num_devices=N_CORES)
    x_d = nc.dram_tensor("x", [ROWS, E], f32, kind="ExternalInput")
    y_d = nc.dram_tensor("y", [BPC * K, E], f32, kind="ExternalOutput")
    y2_d = nc.dram_tensor("y2", [BPC * K, E], f32, kind="ExternalOutput")
    sel_d = nc.dram_tensor("sel", [1, K], u32, kind="ExternalOutput")

    with tile.TileContext(nc) as tc:
        with (
            tc.tile_pool(name="load", bufs=3) as load_pool,
            tc.tile_pool(name="small", bufs=1) as small,
            tc.tile_pool(name="psum", bufs=1, space="PSUM") as psum,
        ):
            # ---- constants (no deps; scheduler fills gaps with these) ----
            ident = small.tile([128, 128], f32)
            make_identity(nc, ident[:])

            # [16, 128] row iota 0..127 (f32) and its >>3 variant
            row_i = small.tile([K, 128], i32)
            nc.gpsimd.iota(out=row_i[:], pattern=[[1, 128]], base=0,
                           channel_multiplier=0)
            row_f = small.tile([K, 128], f32)
            nc.vector.tensor_copy(row_f[:], row_i[:])
            rowd8_i = small.tile([K, 128], i32)
            nc.vector.tensor_scalar(out=rowd8_i[:], in0=row_i[:], scalar1=3,
                                    scalar2=None, op0=Alu.arith_shift_right)
            rowd8_f = small.tile([K, 128], f32)
            nc.vector.tensor_copy(rowd8_f[:], rowd8_i[:])

            # [16, 1] partition iota (f32)
            col16_i = small.tile([K, 1], i32)
            nc.gpsimd.iota(out=col16_i[:], pattern=[[1, 1]], base=0,
                           channel_multiplier=1)
            col16_f = small.tile([K, 1], f32)
            nc.vector.tensor_copy(col16_f[:], col16_i[:])

            # per-round onehot: oh16[r][j, p] = (p>>3 == 8r + j), used to
            # expand the 16 gather rows (base partition must be 0 for PE)
            oh16 = []
            for r in range(2):
                col8r = small.tile([8, 1], f32, tag=f"col8r{r}")
                nc.vector.tensor_scalar(out=col8r[:], in0=col16_f[0:8, :],
                                        scalar1=float(8 * r), scalar2=None,
                                        op0=Alu.add)
                t = small.tile([8, 128], f32, tag=f"oh16_{r}")
                nc.vector.tensor_scalar(out=t[:], in0=rowd8_f[0:8, :],
                                        scalar1=col8r[:], scalar2=None,
                                        op0=Alu.is_equal)
                oh16.append(t)

            # [128, 1] (p & 7) as f32, for gather sub-row offsets
            pp = small.tile([128, 1], i32)
            nc.gpsimd.iota(out=pp[:], pattern=[[1, 1]], base=0,
                           channel_multiplier=1)
            nc.vector.tensor_scalar(out=pp[:], in0=pp[:], scalar1=GR - 1,
                                    scalar2=None, op0=Alu.bitwise_and)
            a7f = small.tile([128, 1], f32)
            nc.vector.tensor_copy(a7f[:], pp[:])

            # rank columns for scatter offsets: rk_f[j, r] = 8r + j + K - OOB
            rk_f = small.tile([8, 2], f32)
            for r in range(2):
                nc.vector.tensor_scalar(out=rk_f[:, r:r + 1],
                                        in0=col16_f[0:8, :],
                                        scalar1=float(8 * r + K - OOB),
                                        scalar2=None, op0=Alu.add)

            xg = x_d[:].rearrange("r (u e) -> (r u) e", u=GR)
            dma_engines = [nc.sync, nc.scalar]
            state = {"n_dma": 0}

            # resident tiles for sample 1 (one per 128-channel half)
            big0 = small.tile([128, E], f32, tag="big0")
            big1 = small.tile([128, E], f32, tag="big1")
            big = [big0, big1]

            def chunk_lists(s):
                return [CHUNKS_LAST if (s == BPC - 1 and h == 1) else CHUNKS
                        for h in range(2)]

            def emit_loads(s, into_resident):
                """Emit this sample's load DMAs; return the dst APs."""
                dsts = [[], []]
                for h in range(2):
                    base = s * C + h * 128
                    off = 0
                    for cw in chunk_lists(s)[h]:
                        if into_resident:
                            dst = big[h][:, off:off + cw]
                        else:
                            t = load_pool.tile([128, 4096], f32)
                            dst = t[:, 0:cw]
                        eng = dma_engines[state["n_dma"] % 2]
                        state["n_dma"] += 1
                        eng.dma_start(out=dst,
                                      in_=x_d[base:base + 128, off:off + cw])
                        dsts[h].append(dst)
                        off += cw
                return dsts

            def emit_reduces(s, dsts):
                """Per-chunk reduces + per-half negated sums + transpose.
                Returns w_neg [1, 256]."""
                ncols = max(len(cl) for cl in chunk_lists(s))
                partials = small.tile([128, 2 * ncols], f32,
                                      tag=f"partials{s}")
                sums = small.tile([128, 2], f32, tag=f"sums{s}")
                psum_w = psum.tile([1, C], f32, tag=f"psw{s}")
                w_neg = small.tile([1, C], f32, tag=f"wneg{s}")
                for h in range(2):
                    cl = chunk_lists(s)[h]
                    for j, dst in enumerate(dsts[h]):
                        nc.vector.reduce_sum(
                            out=partials[:, h * ncols + j:h * ncols + j + 1],
                            in_=dst, axis=X)
                    nc.vector.reduce_sum(
                        out=sums[:, h:h + 1],
                        in_=partials[:, h * ncols:h * ncols + len(cl)],
                        axis=X, negate=True)
                    nc.tensor.matmul(out=psum_w[:, h * 128:(h + 1) * 128],
                                     lhsT=sums[:, h:h + 1], rhs=ident[:],
                                     start=True, stop=True)
                    nc.vector.tensor_copy(w_neg[:, h * 128:(h + 1) * 128],
                                          psum_w[:, h * 128:(h + 1) * 128])
                return w_neg

            def select16(s, w_neg, per_round=None):
                """Two max8 rounds on -sums -> idx_u [1, 16] u32 (bottom-16
                channel indices, ascending pooled sum).  per_round(r, idx_t8)
                is called after each round with that round's transposed
                indices [8, 1] f32 in SBUF."""
                m1 = small.tile([1, 8], f32, tag=f"m1_{s}")
                m2 = small.tile([1, 8], f32, tag=f"m2_{s}")
                w_rep = small.tile([1, C], f32, tag=f"wrep{s}")
                idx_u = small.tile([1, K], u32, tag=f"idxu{s}")

                def round_tail(r):
                    if per_round is None:
                        return
                    idx_f = small.tile([1, 8], f32, tag=f"idxf{s}_{r}")
                    nc.vector.tensor_copy(idx_f[:],
                                          idx_u[:, 8 * r:8 * r + 8])
                    psum_t = psum.tile([8, 1], f32, tag=f"pst{s}")
                    nc.tensor.matmul(out=psum_t[:], lhsT=idx_f[:],
                                     rhs=ident[0:1, 0:1], start=True,
                                     stop=True)
                    idx_t8 = small.tile([8, 1], f32, tag=f"idxt{s}_{r}")
                    nc.vector.tensor_copy(idx_t8[:], psum_t[:])
                    per_round(r, idx_t8)

                nc.vector.max(out=m1[:], in_=w_neg[:])
                nc.vector.max_index(out=idx_u[:, 0:8], in_max=m1[:],
                                    in_values=w_neg[:])
                nc.vector.match_replace(out=w_rep[:], in_to_replace=m1[:],
                                        in_values=w_neg[:], imm_value=-1e38)
                round_tail(0)
                nc.vector.max(out=m2[:], in_=w_rep[:])
                nc.vector.max_index(out=idx_u[:, 8:16], in_max=m2[:],
                                    in_values=w_rep[:])
                round_tail(1)
                return idx_u

            # ---------------- sample 0: stream through pool ----------------
            dsts0 = emit_loads(0, into_resident=False)
            w0 = emit_reduces(0, dsts0)

            # sample 1 load issues next on sync/scalar (before the
            # gather-dependent sample-0 store)
            dsts1 = emit_loads(1, into_resident=True)

            # ---------------- sample 0: select + gather + store ------------
            # gather-row index per partition p: idx[p>>3]*8 + (p&7)
            psum_g = psum.tile([128, 1], f32, tag="psg")
            grow_i = small.tile([128, 1], i32, tag="growi")
            g = small.tile([128, E // GR], f32, tag="g0")

            def s0_round(r, idx_t8):
                nc.tensor.matmul(out=psum_g[:], lhsT=oh16[r][:],
                                 rhs=idx_t8[:], start=(r == 0),
                                 stop=(r == 1))
                if r == 1:
                    grow_f = small.tile([128, 1], f32, tag="growf")
                    nc.vector.tensor_scalar(out=grow_f[:], in0=psum_g[:],
                                            scalar1=float(GR), scalar2=None,
                                            op0=Alu.mult)
                    nc.vector.tensor_tensor(out=grow_i[:], in0=grow_f[:],
                                            in1=a7f[:], op=Alu.add)
                    nc.gpsimd.indirect_dma_start(
                        out=g[:], out_offset=None, in_=xg,
                        in_offset=IndirectOffsetOnAxis(ap=grow_i[:], axis=0))
                    yv0 = y_d[0:K].rearrange("r (u e) -> (r u) e", u=GR)
                    nc.sync.dma_start(out=yv0, in_=g[:])

            select16(0, w0, per_round=s0_round)

            # ---------------- sample 1: reduces + select + scatter ----------
            w1 = emit_reduces(1, dsts1)

            psum_o0 = psum.tile([128, 1], f32, tag="pso0")
            psum_o1 = psum.tile([128, 1], f32, tag="pso1")
            psum_o = [psum_o0, psum_o1]

            def s1_round(r, idx_t8):
                # accumulate per-half scatter offsets:
                # offs[p] = K + rank(p) - OOB contribution if selected
                for h in range(2):
                    if h == 0:
                        idx_cmp = idx_t8
                    else:
                        idx_cmp = small.tile([8, 1], f32, tag=f"idxsh{r}")
                        nc.vector.tensor_scalar(out=idx_cmp[:], in0=idx_t8[:],
                                                scalar1=-128.0, scalar2=None,
                                                op0=Alu.add)
                    oh = small.tile([8, 128], f32, tag=f"oh{h}_{r}")
                    nc.vector.tensor_scalar(out=oh[:], in0=row_f[0:8, :],
                                            scalar1=idx_cmp[:], scalar2=None,
                                            op0=Alu.is_equal)
                    nc.tensor.matmul(out=psum_o[h][:], lhsT=oh[:],
                                     rhs=rk_f[:, r:r + 1],
                                     start=(r == 0), stop=(r == 1))

            idx_u1 = select16(1, w1, per_round=s1_round)
            nc.scalar.dma_start(out=sel_d[:], in_=idx_u1[:])
            outs = [y_d, y2_d]
            for h in range(2):
                offs = small.tile([128, 1], i32, tag=f"offs{h}")
                nc.vector.tensor_scalar(out=offs[:], in0=psum_o[h][:],
                                        scalar1=float(OOB), scalar2=None,
                                        op0=Alu.add)
                nc.gpsimd.indirect_dma_start(
                    out=outs[h][:],
                    out_offset=IndirectOffsetOnAxis(ap=offs[:], axis=0),
                    in_=big[h][:], in_offset=None,
                    bounds_check=BPC * K - 1, oob_is_err=False)

    nc.compile()
    return nc


def get_nc():
    if "nc" not in _cache:
        _cache["nc"] = _build()
    return _cache["nc"]


def make_in_maps(x: np.ndarray) -> list[dict[str, np.ndarray]]:
    x = np.ascontiguousarray(np.asarray(x, dtype=np.float32))
    assert x.shape == (B, C, H, W)
    return [{"x": x[c * BPC:(c + 1) * BPC].reshape(ROWS, E)}
            for c in range(N_CORES)]


def assemble(results: list[dict[str, np.ndarray]]) -> np.ndarray:
    out = np.empty((B, K, H, W), dtype=np.float32)
    for c in range(N_CORES):
        y = results[c]["y"].reshape(BPC, K, H, W).copy()
        y2 = results[c]["y2"].reshape(BPC, K, H, W)
        sel = results[c]["sel"][0]          # [16] uint32, sample-1 channels
        hi = sel >= 128                      # ranks whose channel is in half 1
        y[1, hi] = y2[1, hi]
        out[c * BPC:(c + 1) * BPC] = y
    return out


def kernel(x: np.ndarray) -> np.ndarray:
    nc = get_nc()
    res = run_bass_kernel_spmd(nc, make_in_maps(x), list(range(N_CORES)))
    return assemble(res.results)


# revision 14
# speedup vs baseline: 1.0831x; 1.0274x over previous
"""Bass/Trainium2 kernel for nn_Channel_attention (bottom-16 channel gather).

reference semantics (per sample b):
    weight = mean(x[b], axis=(H, W))           # [C]
    idx    = argsort(weight)[:16]              # ascending pooled value
    out[b] = x[b, idx]                         # [16, H, W]

Strategy: pure data parallel, B=16 sharded 2 samples per core over 8 cores.
Per core (x shard viewed as [512, 16384] = [(sample, channel), H*W]):

  Sample 0 streams through a rotating pool ([128, 4096] tiles, 2 MiB per
  DMA); DVE reduce per chunk -> per-channel sums -> negate -> PE transpose
  -> two max8/max_index rounds give the bottom-16 channel indices in
  ascending pooled order.  The 16 channels are then re-fetched from HBM
  with one [128, 2048] SWDGE indirect gather (16 ch x 8 sub-rows) and
  stored with one direct DMA; both overlap sample 1's streaming.

  Sample 1 streams into two RESIDENT [128, 16384] SBUF tiles (16 MiB), so
  its data is still on-chip when its selection finishes.  The output is
  then written with two SWDGE indirect SCATTERS (SBUF -> DRAM,
  out_offset = output rank for selected channels, OOB-skipped otherwise),
  avoiding the gather re-read and one DMA hop on the end-of-kernel
  critical path.  The two scatters target two different DRAM tensors
  ("y" and "y2") so no write-write dependency serializes them; the host
  merges rows using the stored selection indices ("sel").  Scatter
  offsets are built per max8 round with an accumulating PE matmul so
  round 1's share of the work hides under round 2's DVE ops.

  Emission order per engine queue is load-bearing: sample 1's load
  issues go to the sync/scalar queues before sample 0's gather-dependent
  store (so the waiting store can't stall load descriptor generation),
  and sample 0's select precedes sample 1's reduces on the Vector queue
  (so sample 0's gather+store run mid-stream, not in the tail).
"""

import sys

if "/opt/trn_rl_repo" not in sys.path:
    sys.path.insert(0, "/opt/trn_rl_repo")

import numpy as np

from concourse import bacc, mybir, tile
from concourse.bass import IndirectOffsetOnAxis
from concourse.bass_utils import run_bass_kernel_spmd
from concourse.masks import make_identity

N_CORES = 8
B, C, H, W = 16, 256, 128, 128
K = 16
BPC = B // N_CORES          # samples per core = 2
E = H * W                   # 16384 elems per channel
GR = 8                      # gather sub-rows per channel (8 x 8KiB)
ROWS = BPC * C              # 512 channel rows per core
OOB = 1024                  # scatter offset for unselected channels

f32 = mybir.dt.float32
i32 = mybir.dt.int32
u32 = mybir.dt.uint32
X = mybir.AxisListType.X
Alu = mybir.AluOpType

# sample 0 streams through a rotating pool: small chunks + deep buffering
# so the buffer-recycle semaphores (issue i+bufs waits reduce i) never
# throttle descriptor generation.  sample 1 loads into resident tiles with
# no recycling, so it uses few, large chunks; its last chunks shrink so
# the final reduce exits quickly after the last load lands.
CHUNKS_S0 = [2048] * 8
CHUNKS_S1H0 = [8192, 8192]
CHUNKS_S1H1 = [8192, 4096, 2048, 1024, 512, 512]

_cache = {}


def _build():
    nc = bacc.Bacc("TRN2", target_bir_lowering=False, debug=False,
                   num_devices=N_CORES)
    x_d = nc.dram_tensor("x", [ROWS, E], f32, kind="ExternalInput")
    y_d = nc.dram_tensor("y", [BPC * K, E], f32, kind="ExternalOutput")
    y2_d = nc.dram_tensor("y2", [BPC * K, E], f32, kind="ExternalOutput")
    sel_d = nc.dram_tensor("sel", [1, K], u32, kind="ExternalOutput")

    with tile.TileContext(nc) as tc:
        with (
            tc.tile_pool(name="load", bufs=7) as load_pool,
            tc.tile_pool(name="small", bufs=1) as small,
            tc.tile_pool(name="psum", bufs=1, space="PSUM") as psum,
        ):
            # ---- constants (no deps; scheduler fills gaps with these) ----
            ident = small.tile([128, 128], f32)
            make_identity(nc, ident[:])

            # [16, 128] row iota 0..127 (f32) and its >>3 variant
            row_i = small.tile([K, 128], i32)
            nc.gpsimd.iota(out=row_i[:], pattern=[[1, 128]], base=0,
                           channel_multiplier=0)
            row_f = small.tile([K, 128], f32)
            nc.vector.tensor_copy(row_f[:], row_i[:])
            rowd8_i = small.tile([K, 128], i32)
            nc.vector.tensor_scalar(out=rowd8_i[:], in0=row_i[:], scalar1=3,
                                    scalar2=None, op0=Alu.arith_shift_right)
            rowd8_f = small.tile([K, 128], f32)
            nc.vector.tensor_copy(rowd8_f[:], rowd8_i[:])

            # [16, 1] partition iota (f32)
            col16_i = small.tile([K, 1], i32)
            nc.gpsimd.iota(out=col16_i[:], pattern=[[1, 1]], base=0,
                           channel_multiplier=1)
            col16_f = small.tile([K, 1], f32)
            nc.vector.tensor_copy(col16_f[:], col16_i[:])

            # per-round onehot: oh16[r][j, p] = (p>>3 == 8r + j), used to
            # expand the 16 gather rows (base partition must be 0 for PE)
            oh16 = []
            for r in range(2):
                col8r = small.tile([8, 1], f32, tag=f"col8r{r}")
                nc.vector.tensor_scalar(out=col8r[:], in0=col16_f[0:8, :],
                                        scalar1=float(8 * r), scalar2=None,
                                        op0=Alu.add)
                t = small.tile([8, 128], f32, tag=f"oh16_{r}")
                nc.vector.tensor_scalar(out=t[:], in0=rowd8_f[0:8, :],
                                        scalar1=col8r[:], scalar2=None,
                                        op0=Alu.is_equal)
                oh16.append(t)

            # [128, 1] (p & 7) as f32, for gather sub-row offsets
            pp = small.tile([128, 1], i32)
            nc.gpsimd.iota(out=pp[:], pattern=[[1, 1]], base=0,
                           channel_multiplier=1)
            nc.vector.tensor_scalar(out=pp[:], in0=pp[:], scalar1=GR - 1,
                                    scalar2=None, op0=Alu.bitwise_and)
            a7f = small.tile([128, 1], f32)
            nc.vector.tensor_copy(a7f[:], pp[:])

            # rank columns for scatter offsets: rk_f[j, r] = 8r + j + K - OOB
            rk_f = small.tile([8, 2], f32)
            for r in range(2):
                nc.vector.tensor_scalar(out=rk_f[:, r:r + 1],
                                        in0=col16_f[0:8, :],
                                        scalar1=float(8 * r + K - OOB),
                                        scalar2=None, op0=Alu.add)

            xg = x_d[:].rearrange("r (u e) -> (r u) e", u=GR)
            dma_engines = [nc.sync, nc.scalar]
            state = {"n_dma": 0}

            # resident tiles for sample 1 (one per 128-channel half)
            big0 = small.tile([128, E], f32, tag="big0")
            big1 = small.tile([128, E], f32, tag="big1")
            big = [big0, big1]

            def chunk_lists(s):
                if s == 0:
                    return [CHUNKS_S0, CHUNKS_S0]
                return [CHUNKS_S1H0, CHUNKS_S1H1]

            def emit_loads(s, into_resident):
                """Emit this sample's load DMAs; return the dst APs."""
                dsts = [[], []]
                for h in range(2):
                    base = s * C + h * 128
                    off = 0
                    for cw in chunk_lists(s)[h]:
                        if into_resident:
                            dst = big[h][:, off:off + cw]
                        else:
                            t = load_pool.tile([128, 2048], f32)
                            dst = t[:, 0:cw]
                        eng = dma_engines[state["n_dma"] % 2]
                        state["n_dma"] += 1
                        eng.dma_start(out=dst,
                                      in_=x_d[base:base + 128, off:off + cw])
                        dsts[h].append(dst)
                        off += cw
                return dsts

            def emit_reduces(s, dsts):
                """Per-chunk reduces + per-half negated sums + transpose.
                Returns w_neg [1, 256]."""
                ncols = max(len(cl) for cl in chunk_lists(s))
                partials = small.tile([128, 2 * ncols], f32,
                                      tag=f"partials{s}")
                sums = small.tile([128, 2], f32, tag=f"sums{s}")
                psum_w = psum.tile([1, C], f32, tag=f"psw{s}")
                w_neg = small.tile([1, C], f32, tag=f"wneg{s}")
                for h in range(2):
                    cl = chunk_lists(s)[h]
                    for j, dst in enumerate(dsts[h]):
                        nc.vector.reduce_sum(
                            out=partials[:, h * ncols + j:h * ncols + j + 1],
                            in_=dst, axis=X)
                    nc.vector.reduce_sum(
                        out=sums[:, h:h + 1],
                        in_=partials[:, h * ncols:h * ncols + len(cl)],
                        axis=X, negate=True)
                    nc.tensor.matmul(out=psum_w[:, h * 128:(h + 1) * 128],
                                     lhsT=sums[:, h:h + 1], rhs=ident[:],
                                     start=True, stop=True)
                    nc.vector.tensor_copy(w_neg[:, h * 128:(h + 1) * 128],
                                          psum_w[:, h * 128:(h + 1) * 128])
                return w_neg

            def select16(s, w_neg, per_round=None):
                """Two max8 rounds on -sums -> idx_u [1, 16] u32 (bottom-16
                channel indices, ascending pooled sum).  per_round(r, idx_t8)
                is called after each round with that round's transposed
                indices [8, 1] f32 in SBUF."""
                m1 = small.tile([1, 8], f32, tag=f"m1_{s}")
                m2 = small.tile([1, 8], f32, tag=f"m2_{s}")
                w_rep = small.tile([1, C], f32, tag=f"wrep{s}")
                idx_u = small.tile([1, K], u32, tag=f"idxu{s}")

                def round_tail(r):
                    if per_round is None:
                        return
                    idx_f = small.tile([1, 8], f32, tag=f"idxf{s}_{r}")
                    nc.vector.tensor_copy(idx_f[:],
                                          idx_u[:, 8 * r:8 * r + 8])
                    psum_t = psum.tile([8, 1], f32, tag=f"pst{s}")
                    nc.tensor.matmul(out=psum_t[:], lhsT=idx_f[:],
                                     rhs=ident[0:1, 0:1], start=True,
                                     stop=True)
                    idx_t8 = small.tile([8, 1], f32, tag=f"idxt{s}_{r}")
                    nc.vector.tensor_copy(idx_t8[:], psum_t[:])
                    per_round(r, idx_t8)

                nc.vector.max(out=m1[:], in_=w_neg[:])
                nc.vector.max_index(out=idx_u[:, 0:8], in_max=m1[:],
                                    in_values=w_neg[:])
                nc.vector.match_replace(out=w_rep[:], in_to_replace=m1[:],
                                        in_values=w_neg[:], imm_value=-1e38)
                round_tail(0)
                nc.vector.max(out=m2[:], in_=w_rep[:])
                nc.vector.max_index(out=idx_u[:, 8:16], in_max=m2[:],
                                    in_values=w_rep[:])
                round_tail(1)
                return idx_u

            # ---------------- sample 0: stream through pool ----------------
            dsts0 = emit_loads(0, into_resident=False)
            w0 = emit_reduces(0, dsts0)

            # sample 1 load issues next on sync/scalar (before the
            # gather-dependent sample-0 store)
            dsts1 = emit_loads(1, into_resident=True)

            # ---------------- sample 0: select + gather + store ------------
            # gather-row index per partition p: idx[p>>3]*8 + (p&7)
            psum_g = psum.tile([128, 1], f32, tag="psg")
            grow_i = small.tile([128, 1], i32, tag="growi")
            g = small.tile([128, E // GR], f32, tag="g0")

            def s0_round(r, idx_t8):
                nc.tensor.matmul(out=psum_g[:], lhsT=oh16[r][:],
                                 rhs=idx_t8[:], start=(r == 0),
                                 stop=(r == 1))
                if r == 1:
                    grow_f = small.tile([128, 1], f32, tag="growf")
                    nc.vector.tensor_scalar(out=grow_f[:], in0=psum_g[:],
                                            scalar1=float(GR), scalar2=None,
                                            op0=Alu.mult)
                    nc.vector.tensor_tensor(out=grow_i[:], in0=grow_f[:],
                                            in1=a7f[:], op=Alu.add)
                    nc.gpsimd.indirect_dma_start(
                        out=g[:], out_offset=None, in_=xg,
                        in_offset=IndirectOffsetOnAxis(ap=grow_i[:], axis=0))
                    yv0 = y_d[0:K].rearrange("r (u e) -> (r u) e", u=GR)
                    nc.sync.dma_start(out=yv0, in_=g[:])

            select16(0, w0, per_round=s0_round)

            # ---------------- sample 1: reduces + select + scatter ----------
            w1 = emit_reduces(1, dsts1)

            psum_o0 = psum.tile([128, 1], f32, tag="pso0")
            psum_o1 = psum.tile([128, 1], f32, tag="pso1")
            psum_o = [psum_o0, psum_o1]

            def s1_round(r, idx_t8):
                # accumulate per-half scatter offsets:
                # offs[p] = K + rank(p) - OOB contribution if selected
                for h in range(2):
                    if h == 0:
                        idx_cmp = idx_t8
                    else:
                        idx_cmp = small.tile([8, 1], f32, tag=f"idxsh{r}")
                        nc.vector.tensor_scalar(out=idx_cmp[:], in0=idx_t8[:],
                                                scalar1=-128.0, scalar2=None,
                                                op0=Alu.add)
                    oh = small.tile([8, 128], f32, tag=f"oh{h}_{r}")
                    nc.vector.tensor_scalar(out=oh[:], in0=row_f[0:8, :],
                                            scalar1=idx_cmp[:], scalar2=None,
                                            op0=Alu.is_equal)
                    nc.tensor.matmul(out=psum_o[h][:], lhsT=oh[:],
                                     rhs=rk_f[:, r:r + 1],
                                     start=(r == 0), stop=(r == 1))

            idx_u1 = select16(1, w1, per_round=s1_round)
            nc.scalar.dma_start(out=sel_d[:], in_=idx_u1[:])
            outs = [y_d, y2_d]
            for h in range(2):
                offs = small.tile([128, 1], i32, tag=f"offs{h}")
                nc.vector.tensor_scalar(out=offs[:], in0=psum_o[h][:],
                                        scalar1=float(OOB), scalar2=None,
                                        op0=Alu.add)
                nc.gpsimd.indirect_dma_start(
                    out=outs[h][:],
                    out_offset=IndirectOffsetOnAxis(ap=offs[:], axis=0),
                    in_=big[h][:], in_offset=None,
                    bounds_check=BPC * K - 1, oob_is_err=False)

    nc.compile()
    return nc


def get_nc():
    if "nc" not in _cache:
        _cache["nc"] = _build()
    return _cache["nc"]


def make_in_maps(x: np.ndarray) -> list[dict[str, np.ndarray]]:
    x = np.ascontiguousarray(np.asarray(x, dtype=np.float32))
    assert x.shape == (B, C, H, W)
    return [{"x": x[c * BPC:(c + 1) * BPC].reshape(ROWS, E)}
            for c in range(N_CORES)]


def assemble(results: list[dict[str, np.ndarray]]) -> np.ndarray:
    out = np.empty((B, K, H, W), dtype=np.float32)
    for c in range(N_CORES):
        y = results[c]["y"].reshape(BPC, K, H, W).copy()
        y2 = results[c]["y2"].reshape(BPC, K, H, W)
        sel = results[c]["sel"][0]          # [16] uint32, sample-1 channels
        hi = sel >= 128                      # ranks whose channel is in half 1
        y[1, hi] = y2[1, hi]
        out[c * BPC:(c + 1) * BPC] = y
    return out


def kernel(x: np.ndarray) -> np.ndarray:
    nc = get_nc()
    res = run_bass_kernel_spmd(nc, make_in_maps(x), list(range(N_CORES)))
    return assemble(res.results)


# revision 15
# speedup vs baseline: 1.2849x; 1.1863x over previous
"""Bass/Trainium2 kernel for nn_Channel_attention (bottom-16 channel gather).

reference semantics (per sample b):
    weight = mean(x[b], axis=(H, W))           # [C]
    idx    = argsort(weight)[:16]              # ascending pooled value
    out[b] = x[b, idx]                         # [16, H, W]

Strategy: pure data parallel, B=16 sharded 2 samples per core over 8 cores.
Per core (x shard viewed as [512, 16384] = [(sample, channel), H*W]):

  Sample 0 streams into two RESIDENT [128, 16384] SBUF tiles (16 MiB).
  None of its ops ever wait on a buffer-recycle semaphore, so the Tile
  scheduler keeps its load -> reduce -> select chain first and the DMA
  queues are never head-of-line blocked.  Its output is written mid-run
  with two SWDGE indirect SCATTERS (SBUF -> DRAM, out_offset = output
  rank for selected channels, OOB-skipped otherwise), which avoids the
  1 MiB gather re-read entirely.  The two scatters target two different
  DRAM tensors ("y" and "y2") so no write-write dependency serializes
  them; the host merges rows using the stored selection indices ("sel").
  Scatter offsets are built per max8 round with an accumulating PE
  matmul so round 1's share of the work hides under round 2's DVE ops.

  Sample 1 streams through a rotating pool ([128, 2048] tiles, bufs=7);
  by the time its buffer-recycle semaphores matter, Vector has nothing
  queued but sample-1 reduces, so the recycling never throttles DMA.
  Its selection finishes last, so its 16 channels are re-fetched with
  two full-width [128, 1024] SWDGE indirect gathers (one per max8
  round, 8 ch x 16 sub-rows each; round 1's gather+store overlaps round
  2's select) and stored with direct DMAs.  The last half's chunks
  shrink so the final reduce exits quickly after the last load lands.
"""

import sys

if "/opt/trn_rl_repo" not in sys.path:
    sys.path.insert(0, "/opt/trn_rl_repo")

import numpy as np

from concourse import bacc, mybir, tile
from concourse.bass import IndirectOffsetOnAxis
from concourse.bass_utils import run_bass_kernel_spmd
from concourse.masks import make_identity

N_CORES = 8
B, C, H, W = 16, 256, 128, 128
K = 16
BPC = B // N_CORES          # samples per core = 2
E = H * W                   # 16384 elems per channel
GR = 16                     # gather sub-rows per channel (16 x 4KiB)
ROWS = BPC * C              # 512 channel rows per core
OOB = 1024                  # scatter offset for unselected channels

f32 = mybir.dt.float32
i32 = mybir.dt.int32
u32 = mybir.dt.uint32
X = mybir.AxisListType.X
Alu = mybir.AluOpType

CHUNKS_S0 = [4096] * 4                                # resident, unthrottled
CHUNKS_S1H0 = [2048] * 8                              # pooled
CHUNKS_S1H1 = [2048] * 7 + [1024, 512, 512]           # small tail chunks

_cache = {}


def _build():
    nc = bacc.Bacc("TRN2", target_bir_lowering=False, debug=False,
                   num_devices=N_CORES)
    x_d = nc.dram_tensor("x", [ROWS, E], f32, kind="ExternalInput")
    y_d = nc.dram_tensor("y", [BPC * K, E], f32, kind="ExternalOutput")
    y2_d = nc.dram_tensor("y2", [BPC * K, E], f32, kind="ExternalOutput")
    sel_d = nc.dram_tensor("sel", [1, K], u32, kind="ExternalOutput")

    with tile.TileContext(nc) as tc:
        with (
            tc.tile_pool(name="load", bufs=7) as load_pool,
            tc.tile_pool(name="small", bufs=1) as small,
            tc.tile_pool(name="psum", bufs=1, space="PSUM") as psum,
        ):
            # ---- constants (no deps; scheduler fills gaps with these) ----
            ident = small.tile([128, 128], f32)
            make_identity(nc, ident[:])

            # [8, 128] row iota 0..127 (f32) and (p>>4) variant
            row_i = small.tile([8, 128], i32)
            nc.gpsimd.iota(out=row_i[:], pattern=[[1, 128]], base=0,
                           channel_multiplier=0)
            row_f = small.tile([8, 128], f32)
            nc.vector.tensor_copy(row_f[:], row_i[:])
            rowd16_i = small.tile([8, 128], i32)
            nc.vector.tensor_scalar(out=rowd16_i[:], in0=row_i[:], scalar1=4,
                                    scalar2=None, op0=Alu.arith_shift_right)
            rowd16_f = small.tile([8, 128], f32)
            nc.vector.tensor_copy(rowd16_f[:], rowd16_i[:])

            # [8, 1] partition iota (f32)
            col8_i = small.tile([8, 1], i32)
            nc.gpsimd.iota(out=col8_i[:], pattern=[[1, 1]], base=0,
                           channel_multiplier=1)
            col8_f = small.tile([8, 1], f32)
            nc.vector.tensor_copy(col8_f[:], col8_i[:])

            # oh8_16[j, p] = (p>>4 == j): expands 8 ranks to 128 gather rows
            oh8_16 = small.tile([8, 128], f32)
            nc.vector.tensor_scalar(out=oh8_16[:], in0=rowd16_f[:],
                                    scalar1=col8_f[:], scalar2=None,
                                    op0=Alu.is_equal)

            # [128, 1] (p & 15) as f32, for gather sub-row offsets
            pp = small.tile([128, 1], i32)
            nc.gpsimd.iota(out=pp[:], pattern=[[1, 1]], base=0,
                           channel_multiplier=1)
            nc.vector.tensor_scalar(out=pp[:], in0=pp[:], scalar1=GR - 1,
                                    scalar2=None, op0=Alu.bitwise_and)
            a15f = small.tile([128, 1], f32)
            nc.vector.tensor_copy(a15f[:], pp[:])

            # rank columns for scatter offsets: rk_f[j, r] = 8r + j - OOB
            rk_f = small.tile([8, 2], f32)
            for r in range(2):
                nc.vector.tensor_scalar(out=rk_f[:, r:r + 1], in0=col8_f[:],
                                        scalar1=float(8 * r - OOB),
                                        scalar2=None, op0=Alu.add)

            xg = x_d[:].rearrange("r (u e) -> (r u) e", u=GR)
            dma_engines = [nc.sync, nc.scalar]
            state = {"n_dma": 0}

            # resident tiles for sample 0 (one per 128-channel half)
            big0 = small.tile([128, E], f32, tag="big0")
            big1 = small.tile([128, E], f32, tag="big1")
            big = [big0, big1]

            def chunk_lists(s):
                if s == 0:
                    return [CHUNKS_S0, CHUNKS_S0]
                return [CHUNKS_S1H0, CHUNKS_S1H1]

            def emit_loads(s, into_resident):
                dsts = [[], []]
                for h in range(2):
                    base = s * C + h * 128
                    off = 0
                    for cw in chunk_lists(s)[h]:
                        if into_resident:
                            dst = big[h][:, off:off + cw]
                        else:
                            t = load_pool.tile([128, 2048], f32)
                            dst = t[:, 0:cw]
                        eng = dma_engines[state["n_dma"] % 2]
                        state["n_dma"] += 1
                        eng.dma_start(out=dst,
                                      in_=x_d[base:base + 128, off:off + cw])
                        dsts[h].append(dst)
                        off += cw
                return dsts

            def emit_reduces(s, dsts):
                ncols = max(len(cl) for cl in chunk_lists(s))
                partials = small.tile([128, 2 * ncols], f32,
                                      tag=f"partials{s}")
                sums = small.tile([128, 2], f32, tag=f"sums{s}")
                psum_w = psum.tile([1, C], f32, tag=f"psw{s}")
                w_neg = small.tile([1, C], f32, tag=f"wneg{s}")
                for h in range(2):
                    cl = chunk_lists(s)[h]
                    for j, dst in enumerate(dsts[h]):
                        nc.vector.reduce_sum(
                            out=partials[:, h * ncols + j:h * ncols + j + 1],
                            in_=dst, axis=X)
                    nc.vector.reduce_sum(
                        out=sums[:, h:h + 1],
                        in_=partials[:, h * ncols:h * ncols + len(cl)],
                        axis=X, negate=True)
                    nc.tensor.matmul(out=psum_w[:, h * 128:(h + 1) * 128],
                                     lhsT=sums[:, h:h + 1], rhs=ident[:],
                                     start=True, stop=True)
                    nc.vector.tensor_copy(w_neg[:, h * 128:(h + 1) * 128],
                                          psum_w[:, h * 128:(h + 1) * 128])
                return w_neg

            def select16(s, w_neg, per_round):
                """Two max8 rounds on -sums.  per_round(r, idx_t8) runs after
                each round with that round's transposed indices [8, 1] f32.
                Returns idx_u [1, 16] u32 (ascending pooled sum)."""
                m1 = small.tile([1, 8], f32, tag=f"m1_{s}")
                m2 = small.tile([1, 8], f32, tag=f"m2_{s}")
                w_rep = small.tile([1, C], f32, tag=f"wrep{s}")
                idx_u = small.tile([1, K], u32, tag=f"idxu{s}")

                def round_tail(r):
                    idx_f = small.tile([1, 8], f32, tag=f"idxf{s}_{r}")
                    nc.vector.tensor_copy(idx_f[:],
                                          idx_u[:, 8 * r:8 * r + 8])
                    psum_t = psum.tile([8, 1], f32, tag=f"pst{s}")
                    nc.tensor.matmul(out=psum_t[:], lhsT=idx_f[:],
                                     rhs=ident[0:1, 0:1], start=True,
                                     stop=True)
                    idx_t8 = small.tile([8, 1], f32, tag=f"idxt{s}_{r}")
                    nc.vector.tensor_copy(idx_t8[:], psum_t[:])
                    per_round(r, idx_t8)

                nc.vector.max(out=m1[:], in_=w_neg[:])
                nc.vector.max_index(out=idx_u[:, 0:8], in_max=m1[:],
                                    in_values=w_neg[:])
                nc.vector.match_replace(out=w_rep[:], in_to_replace=m1[:],
                                        in_values=w_neg[:], imm_value=-1e38)
                round_tail(0)
                nc.vector.max(out=m2[:], in_=w_rep[:])
                nc.vector.max_index(out=idx_u[:, 8:16], in_max=m2[:],
                                    in_values=w_rep[:])
                round_tail(1)
                return idx_u

            # ------------- sample 0: resident loads + reduces ---------------
            dsts0 = emit_loads(0, into_resident=True)
            w0 = emit_reduces(0, dsts0)

            # sample 1 pooled load issues queue next on sync/scalar
            dsts1 = emit_loads(1, into_resident=False)

            # ------------- sample 0: select + mid-run scatters --------------
            psum_o0 = psum.tile([128, 1], f32, tag="pso0")
            psum_o1 = psum.tile([128, 1], f32, tag="pso1")
            psum_o = [psum_o0, psum_o1]

            def s0_round(r, idx_t8):
                # accumulate per-half scatter offsets:
                # offs[p] = rank(p) - OOB contribution if channel p selected
                for h in range(2):
                    if h == 0:
                        idx_cmp = idx_t8
                    else:
                        idx_cmp = small.tile([8, 1], f32, tag=f"idxsh{r}")
                        nc.vector.tensor_scalar(out=idx_cmp[:], in0=idx_t8[:],
                                                scalar1=-128.0, scalar2=None,
                                                op0=Alu.add)
                    oh = small.tile([8, 128], f32, tag=f"oh{h}_{r}")
                    nc.vector.tensor_scalar(out=oh[:], in0=row_f[:],
                                            scalar1=idx_cmp[:], scalar2=None,
                                            op0=Alu.is_equal)
                    nc.tensor.matmul(out=psum_o[h][:], lhsT=oh[:],
                                     rhs=rk_f[:, r:r + 1],
                                     start=(r == 0), stop=(r == 1))

            idx_u0 = select16(0, w0, per_round=s0_round)
            nc.scalar.dma_start(out=sel_d[:], in_=idx_u0[:])
            outs = [y_d, y2_d]
            for h in range(2):
                offs = small.tile([128, 1], i32, tag=f"offs{h}")
                nc.vector.tensor_scalar(out=offs[:], in0=psum_o[h][:],
                                        scalar1=float(OOB), scalar2=None,
                                        op0=Alu.add)
                nc.gpsimd.indirect_dma_start(
                    out=outs[h][:],
                    out_offset=IndirectOffsetOnAxis(ap=offs[:], axis=0),
                    in_=big[h][:], in_offset=None,
                    bounds_check=BPC * K - 1, oob_is_err=False)

            # ------------- sample 1: reduces + select + gather/store --------
            w1 = emit_reduces(1, dsts1)

            yv1 = y_d[K:2 * K].rearrange("r (u e) -> (r u) e", u=GR)

            def s1_round(r, idx_t8):
                # gather-row index per partition p (rank k = 8r + (p>>4)):
                # (C + idx[k])*16 + (p&15)
                psum_g = psum.tile([128, 1], f32, tag=f"psg{r}")
                nc.tensor.matmul(out=psum_g[:], lhsT=oh8_16[:], rhs=idx_t8[:],
                                 start=True, stop=True)
                grow_f = small.tile([128, 1], f32, tag=f"growf{r}")
                nc.vector.tensor_scalar(out=grow_f[:], in0=psum_g[:],
                                        scalar1=float(GR),
                                        scalar2=float(C * GR), op0=Alu.mult,
                                        op1=Alu.add)
                grow_i = small.tile([128, 1], i32, tag=f"growi{r}")
                nc.vector.tensor_tensor(out=grow_i[:], in0=grow_f[:],
                                        in1=a15f[:], op=Alu.add)
                g = small.tile([128, E // GR], f32, tag=f"g{r}")
                nc.gpsimd.indirect_dma_start(
                    out=g[:], out_offset=None, in_=xg,
                    in_offset=IndirectOffsetOnAxis(ap=grow_i[:], axis=0))
                dma_engines[r].dma_start(out=yv1[128 * r:128 * (r + 1), :],
                                         in_=g[:])

            select16(1, w1, per_round=s1_round)

    nc.compile()
    return nc


def get_nc():
    if "nc" not in _cache:
        _cache["nc"] = _build()
    return _cache["nc"]


def make_in_maps(x: np.ndarray) -> list[dict[str, np.ndarray]]:
    x = np.ascontiguousarray(np.asarray(x, dtype=np.float32))
    assert x.shape == (B, C, H, W)
    return [{"x": x[c * BPC:(c + 1) * BPC].reshape(ROWS, E)}
            for c in range(N_CORES)]


def assemble(results: list[dict[str, np.ndarray]]) -> np.ndarray:
    out = np.empty((B, K, H, W), dtype=np.float32)
    for c in range(N_CORES):
        y = results[c]["y"].reshape(BPC, K, H, W).copy()
        y2 = results[c]["y2"].reshape(BPC, K, H, W)
        sel = results[c]["sel"][0]          # [16] uint32, sample-0 channels
        hi = sel >= 128                      # ranks whose channel is in half 1
        y[0, hi] = y2[0, hi]
        out[c * BPC:(c + 1) * BPC] = y
    return out


def kernel(x: np.ndarray) -> np.ndarray:
    nc = get_nc()
    res = run_bass_kernel_spmd(nc, make_in_maps(x), list(range(N_CORES)))
    return assemble(res.results)
